# revision 8
# baseline (speedup 1.0000x reference)
# DCPLoss Trainium2 Bass kernel.
#
# Computation (see nn_DCPLoss): dark-channel prior loss over [1,8,3,640,640]
# f32 inputs gt/output/output_ema. Sharding: spatial over H across 8 cores
# (80 rows each + reflected halos prepared on host). No collectives: each
# core returns row-partial sums; the host finishes the (tiny) scalar math.
#
# Exactness tricks (all validated on HW):
#  - floor(255*x) commutes with min, so channel mins run on raw f32 first.
#  - rtz-to-bf16 of 255*m (AND-mask of low mantissa bits) preserves
#    floor(min(.)) exactly while enabling bf16 2x-mode min ops on DVE.
#  - exact floor at the end via int16 convert (RNE on HW) of v - (0.5 - 2^-16):
#    exact for every reachable v in [0, 256) (v has <= 8 significand bits).
#  - host reflect-padding makes border handling uniform: reflected input rows
#    reproduce both cv2's +inf-border erosion and jnp.pad reflect exactly.
#
# Perf structure: frames processed in PAIRS through the vertical-min chain
# (halves the SBUF->SBUF shift-DMA count and the DVE/ACT instruction count
# there — per-DMA sequencer dispatch is a dominant serial cost). Work is
# spread over DVE (mins/subs), ACT (scale/abs/floor), GpSimd (accumulator
# adds + S path), PE (banded-ones matmul for vertical box sums).
import sys
import numpy as np

sys.path.insert(0, "/opt/trn_rl_repo")

from contextlib import ExitStack

import concourse.bass as bass
import concourse.tile as tile
from concourse import bacc, mybir, bass_utils

A = mybir.AluOpType
F = mybir.ActivationFunctionType
DT = mybir.dt

NCORES = 8
H = W = 640
T = 8           # frames
C = 3           # channels
NT = 3          # tensors: gt, output, ema
ROWS = H // NCORES          # 80 output rows per core
HALO = 10                   # 7 (erosion) + 3 (local var)
RIN = ROWS + 2 * HALO       # 100 input rows per core
WIN = W + 2 * HALO          # 660 input cols
DCW = W + 6                 # 646 dc cols ([-3, 643))
DCR = ROWS + 6              # 86 dc rows
FLOOR_BIAS = -(0.5 - 2.0 ** -16)
GP_OFFLOAD = True   # run accumulation adds + S-sub on GpSimd to unload DVE

_CACHE = {}


def _build_nc():
    nc = bacc.Bacc("TRN2", target_bir_lowering=False, debug=False)
    fb = nc.alloc_sbuf_tensor("const_floorbias", [128, 1], DT.float32)
    nc.gpsimd.memset(fb.ap(), FLOOR_BIAS)
    nc.all_engine_barrier()
    xin = nc.dram_tensor("xin", [T, NT, C, RIN, WIN], DT.float32, kind="ExternalInput")
    band = nc.dram_tensor("band", [DCR, ROWS], DT.float32, kind="ExternalInput")
    out = nc.dram_tensor("out", [ROWS, 4], DT.float32, kind="ExternalOutput")

    with tile.TileContext(nc) as tc, ExitStack() as ctx:
        def pool(nm, bufs=2):
            return ctx.enter_context(tc.tile_pool(name=nm, bufs=bufs))

        p_x3 = pool("x3")
        p_m = pool("m", bufs=1)
        p_y = pool("y", bufs=1)
        p_e = pool("e", bufs=1)
        p_h = pool("h", bufs=1)
        p_v = pool("v", bufs=1)
        p_hmv = pool("hmv", bufs=2)
        p_vs = pool("vs", bufs=2)
        p_dc = pool("dc", bufs=1)
        p_acc = pool("acc", bufs=1)
        p_ds = pool("ds", bufs=1)
        p_fin = pool("fin", bufs=1)

        racc_sr = p_acc.tile([DCR, DCW], DT.float32, name="racc_sr", tag="racc_sr")
        racc_em = p_acc.tile([DCR, DCW], DT.float32, name="racc_em", tag="racc_em")
        smap = p_acc.tile([RIN, W], DT.float32, name="smap", tag="smap")

        L = NT * WIN
        PW2 = NT * DCW
        eng_acc = nc.gpsimd if GP_OFFLOAD else nc.vector

        for tp in range(T // 2):
          # compact pair tile: [f0(gt,out,ema), f1(gt,out,ema)] x 646 cols
          hmv = p_hmv.tile([RIN, 2 * PW2], DT.bfloat16, name=f"hmv_{tp}", tag="hmv")
          for tf in range(2):
            t = 2 * tp + tf
            # ---- load: one DMA per frame, x3 [RIN, NT*C*WIN] ----
            x3 = p_x3.tile([RIN, NT * C * WIN], DT.float32, name=f"x3_{t}", tag="x3")
            nc.sync.dma_start(
                x3[:].rearrange("p (q w) -> p q w", q=NT * C),
                xin.ap()[t].rearrange("n c p w -> p (n c) w"),
            )
            x3v = x3[:].rearrange("p (n c w) -> p n c w", n=NT, c=C)

            # ---- channel min (raw f32; floor/scale commute with min) ----
            m = p_m.tile([RIN, L], DT.float32, name=f"m_{t}", tag="m")
            mv = m[:].rearrange("p (n w) -> p n w", n=NT)
            nc.vector.tensor_tensor(mv, x3v[:, :, 0, :], x3v[:, :, 1, :], A.min)
            nc.vector.tensor_tensor(mv, mv, x3v[:, :, 2, :], A.min)

            # ---- quantize: e = rtz_bf16(255*m), exact ----
            y = p_y.tile([RIN, L], DT.float32, name=f"y_{t}", tag="y")
            nc.scalar.activation(y[:], m[:], F.Copy, scale=255.0)
            nc.vector.tensor_scalar(
                y[:].bitcast(DT.uint32), y[:].bitcast(DT.uint32),
                0xFFFF0000, None, A.bitwise_and)
            e = p_e.tile([RIN, L], DT.bfloat16, name=f"e_{t}", tag="e")
            nc.scalar.activation(e[:], y[:], F.Copy)

            # ---- horizontal window-15 min tree (free-dim shifts) ----
            h1 = p_h.tile([RIN, L], DT.bfloat16, name=f"h1_{t}", tag="h1")
            nc.vector.tensor_tensor(h1[:, 0:L - 1], e[:, 0:L - 1], e[:, 1:L], A.min)
            h2 = p_h.tile([RIN, L], DT.bfloat16, name=f"h2_{t}", tag="h2")
            nc.vector.tensor_tensor(h2[:, 0:L - 3], h1[:, 0:L - 3], h1[:, 2:L - 1], A.min)
            nc.vector.tensor_tensor(h1[:, 0:L - 7], h2[:, 0:L - 7], h2[:, 4:L - 3], A.min)
            # final level: write compact dc cols into the pair tile
            h1v = h1[:].rearrange("p (n w) -> p n w", n=NT)
            nc.vector.tensor_tensor(
                hmv[:, tf * PW2:(tf + 1) * PW2].rearrange("p (n w) -> p n w", n=NT),
                h1v[:, :, 0:DCW], h1v[:, :, 7:7 + DCW], A.min)

            # ---- S map: smap += sum_c |out - gt| (raw f32, all RIN rows) ----
            eng_s = nc.gpsimd if GP_OFFLOAD else nc.vector
            ds = p_ds.tile([RIN, C * W], DT.float32, name=f"ds_{t}", tag="ds")
            dsv = ds[:].rearrange("p (c w) -> p c w", c=C)
            eng_s.tensor_tensor(
                dsv,
                x3v[:, 1, :, HALO:HALO + W],
                x3v[:, 0, :, HALO:HALO + W],
                A.subtract)
            ads = p_ds.tile([RIN, C * W], DT.float32, name=f"ads_{t}", tag="ads")
            nc.scalar.activation(ads[:], ds[:], F.Abs)
            for c in range(C):
                acol = ads[:, c * W:(c + 1) * W]
                if t == 0 and c == 0:
                    eng_s.tensor_copy(smap[:], acol)
                else:
                    eng_s.tensor_tensor(smap[:], smap[:], acol, A.add)

          # ---- pair-batched vertical window-15 min tree (DMA shifts) ----
          DL = 2 * PW2
          vs1 = p_vs.tile([RIN, DL], DT.bfloat16, name=f"vs1_{tp}", tag="vs")
          nc.sync.dma_start(vs1[0:RIN - 1], hmv[1:RIN])
          v1 = p_v.tile([RIN, DL], DT.bfloat16, name=f"v1_{tp}", tag="v1")
          nc.vector.tensor_tensor(v1[0:RIN - 1], hmv[0:RIN - 1], vs1[0:RIN - 1], A.min)
          vs2 = p_vs.tile([RIN, DL], DT.bfloat16, name=f"vs2_{tp}", tag="vs")
          nc.gpsimd.dma_start(vs2[0:RIN - 3], v1[2:RIN - 1])
          v2 = p_v.tile([RIN, DL], DT.bfloat16, name=f"v2_{tp}", tag="v2")
          nc.vector.tensor_tensor(v2[0:RIN - 3], v1[0:RIN - 3], vs2[0:RIN - 3], A.min)
          vs3 = p_vs.tile([RIN, DL], DT.bfloat16, name=f"vs3_{tp}", tag="vs")
          nc.scalar.dma_start(vs3[0:RIN - 7], v2[4:RIN - 3])
          v3 = p_v.tile([RIN, DL], DT.bfloat16, name=f"v3_{tp}", tag="v1")
          nc.vector.tensor_tensor(v3[0:RIN - 7], v2[0:RIN - 7], vs3[0:RIN - 7], A.min)
          vs4 = p_vs.tile([RIN, DL], DT.bfloat16, name=f"vs4_{tp}", tag="vs")
          nc.gpsimd.dma_start(vs4[0:DCR], v3[7:RIN - 7])
          v4 = p_v.tile([DCR, DL], DT.bfloat16, name=f"v4_{tp}", tag="v2")
          nc.vector.tensor_tensor(v4[:], v3[0:DCR], vs4[0:DCR], A.min)

          # ---- exact floor -> int16 dark channels (255 units), both frames ----
          dc = p_dc.tile([DCR, DL], DT.int16, name=f"dc_{tp}", tag="dc")
          nc.scalar.activation(dc[:], v4[:], F.Identity, bias=fb.ap()[:DCR], scale=1.0)

          # ---- residuals (pair-batched): racc += |dc_gt - dc_x| ----
          dcv = dc[:].rearrange("p (f n w) -> p f n w", f=2, n=NT)
          d1 = p_ds.tile([DCR, 2 * DCW], DT.int16, name=f"d1_{tp}", tag="d1")
          nc.vector.tensor_tensor(d1[:].rearrange("p (f w) -> p f w", f=2),
                                  dcv[:, :, 0, :], dcv[:, :, 1, :], A.subtract)
          d2 = p_ds.tile([DCR, 2 * DCW], DT.int16, name=f"d2_{tp}", tag="d2")
          nc.vector.tensor_tensor(d2[:].rearrange("p (f w) -> p f w", f=2),
                                  dcv[:, :, 0, :], dcv[:, :, 2, :], A.subtract)
          ad1 = p_ds.tile([DCR, 2 * DCW], DT.int16, name=f"ad1_{tp}", tag="ad1")
          nc.scalar.activation(ad1[:], d1[:], F.Abs)
          ad2 = p_ds.tile([DCR, 2 * DCW], DT.int16, name=f"ad2_{tp}", tag="ad2")
          nc.scalar.activation(ad2[:], d2[:], F.Abs)
          if tp == 0:
              nc.vector.tensor_copy(racc_sr[:], ad1[:, 0:DCW])
              nc.vector.tensor_copy(racc_em[:], ad2[:, 0:DCW])
          else:
              eng_acc.tensor_tensor(racc_sr[:], racc_sr[:], ad1[:, 0:DCW], A.add)
              eng_acc.tensor_tensor(racc_em[:], racc_em[:], ad2[:, 0:DCW], A.add)
          eng_acc.tensor_tensor(racc_sr[:], racc_sr[:], ad1[:, DCW:2 * DCW], A.add)
          eng_acc.tensor_tensor(racc_em[:], racc_em[:], ad2[:, DCW:2 * DCW], A.add)

        # ================= final stage (once per core) =================
        # racc_sr: [86, 646] f32 (255-unit ints), rows a-3..b+3, cols -3..643
        sq = p_fin.tile([DCR, DCW], DT.float32, name="sq", tag="sq")
        nc.scalar.activation(sq[:], racc_sr[:], F.Square)

        bandt = p_fin.tile([DCR, ROWS], DT.float32, name="bandt", tag="bandt")
        nc.sync.dma_start(bandt[:], band.ap())

        p_ps = ctx.enter_context(tc.tile_pool(name="psum", bufs=1, space="PSUM"))

        def box7(src, nm):
            t2 = p_fin.tile([DCR, DCW], DT.float32, name=f"{nm}_t2", tag="bx_t2")
            nc.vector.tensor_tensor(
                t2[:, 0:DCW - 1], src[:, 0:DCW - 1], src[:, 1:DCW], A.add)
            t4 = p_fin.tile([DCR, DCW], DT.float32, name=f"{nm}_t4", tag="bx_t4")
            nc.vector.tensor_tensor(
                t4[:, 0:DCW - 3], t2[:, 0:DCW - 3], t2[:, 2:DCW - 1], A.add)
            t6 = p_fin.tile([DCR, W], DT.float32, name=f"{nm}_t6", tag="bx_t6")
            nc.vector.tensor_tensor(t6[:], t4[:, 0:W], t2[:, 4:4 + W], A.add)
            s7 = p_fin.tile([DCR, W], DT.float32, name=f"{nm}_s7", tag="bx_s7")
            nc.vector.tensor_tensor(s7[:], t6[:], src[:, 6:6 + W], A.add)
            ps = p_ps.tile([ROWS, W], DT.float32, name=f"{nm}_ps", tag=f"{nm}_ps")
            nc.tensor.matmul(ps[:, 0:512], bandt[:], s7[:, 0:512])
            nc.tensor.matmul(ps[:, 512:W], bandt[:], s7[:, 512:W])
            return ps

        ps1 = box7(racc_sr, "b1")
        ps2 = box7(sq, "b2")
        s1sq = p_fin.tile([ROWS, W], DT.float32, name="s1sq", tag="s1sq")
        nc.scalar.activation(s1sq[:], ps1[:], F.Square)
        # pw_un = s2b - s1sq/49  (= 49*48*var scaled; sign kept, abs later)
        pw = p_fin.tile([ROWS, W], DT.float32, name="pw", tag="pw")
        nc.vector.scalar_tensor_tensor(
            pw[:], s1sq[:], -1.0 / 49.0, ps2[:], A.mult, A.add)
        wabs = p_fin.tile([ROWS, W], DT.float32, name="wabs", tag="wabs")
        nc.scalar.activation(wabs[:], pw[:], F.Abs)

        # center-aligned copies (partition offset 3 -> 0 via DMA)
        rsr_c = p_fin.tile([ROWS, DCW], DT.float32, name="rsr_c", tag="rsr_c")
        nc.scalar.dma_start(rsr_c[:], racc_sr[3:3 + ROWS])
        rem_c = p_fin.tile([ROWS, DCW], DT.float32, name="rem_c", tag="rem_c")
        nc.scalar.dma_start(rem_c[:], racc_em[3:3 + ROWS])
        sc = p_fin.tile([ROWS, W], DT.float32, name="sc", tag="sc")
        nc.scalar.dma_start(sc[:], smap[HALO:HALO + ROWS])

        mask = p_fin.tile([ROWS, W], DT.float32, name="mask", tag="mask")
        nc.vector.tensor_tensor(
            mask[:], rsr_c[:, 3:3 + W], rem_c[:, 3:3 + W], A.is_ge)
        nc.vector.tensor_tensor(wabs[:], wabs[:], mask[:], A.mult)

        # row partials
        scr = p_fin.tile([ROWS, W], DT.float32, name="scr", tag="scr")
        ws_r = p_fin.tile([ROWS, 1], DT.float32, name="ws_r", tag="ws_r")
        nc.vector.tensor_tensor(scr[:], wabs[:], sc[:], A.mult)
        nc.vector.tensor_reduce(ws_r[:], scr[:], mybir.AxisListType.X, A.add)
        r1 = p_fin.tile([ROWS, 1], DT.float32, name="r1", tag="r1")
        nc.vector.tensor_reduce(
            r1[:], rsr_c[:, 3:3 + W], mybir.AxisListType.X, A.add)
        scr2 = p_fin.tile([ROWS, W], DT.float32, name="scr2", tag="scr")
        r2 = p_fin.tile([ROWS, 1], DT.float32, name="r2", tag="r2")
        nc.scalar.activation(scr2[:], rsr_c[:, 3:3 + W], F.Square)
        nc.vector.tensor_reduce(r2[:], scr2[:], mybir.AxisListType.X, A.add)

        nc.sync.dma_start(out.ap()[:, 0:1], r1[:])
        nc.sync.dma_start(out.ap()[:, 1:2], r2[:])
        nc.sync.dma_start(out.ap()[:, 2:3], ws_r[:])
        nc.sync.dma_start(out.ap()[:, 3:4], r1[:])

    nc.compile()
    return nc


def _band_matrix():
    b = np.zeros((DCR, ROWS), dtype=np.float32)
    for r in range(ROWS):
        b[r:r + 7, r] = 1.0
    return b


def _prep_inputs(gt, output, output_ema):
    full = np.stack([
        np.asarray(gt)[0], np.asarray(output)[0], np.asarray(output_ema)[0]
    ]).astype(np.float32)                       # [3, 8, 3, 640, 640]
    padded = np.pad(full, ((0, 0), (0, 0), (0, 0), (HALO, HALO), (HALO, HALO)),
                    mode="reflect")             # [3, 8, 3, 660, 660]
    band = _band_matrix()
    in_maps = []
    for i in range(NCORES):
        slab = np.ascontiguousarray(
            padded[:, :, :, ROWS * i:ROWS * i + RIN, :].transpose(1, 0, 2, 3, 4))
        in_maps.append({"xin": slab, "band": band})
    return in_maps


def _host_finish(outs):
    r1 = sum(float(o["out"][:, 0].astype(np.float64).sum()) for o in outs)
    r2 = sum(float(o["out"][:, 1].astype(np.float64).sum()) for o in outs)
    ws = sum(float(o["out"][:, 2].astype(np.float64).sum()) for o in outs)
    n = float(H * W)
    var_u = (r2 - r1 * r1 / n) / (n - 1.0) / (255.0 ** 2)
    patch_w = var_u ** 0.2
    ntot = float(T * C * H * W)
    loss = patch_w * ws / (48.0 * 255.0 ** 2) / ntot
    return np.float32(loss)


def kernel(**inputs):
    if "nc" not in _CACHE:
        _CACHE["nc"] = _build_nc()
    nc = _CACHE["nc"]
    in_maps = _prep_inputs(inputs["gt"], inputs["output"], inputs["output_ema"])
    res = bass_utils.run_bass_kernel_spmd(nc, in_maps, core_ids=list(range(NCORES)))
    return _host_finish(res.results)
